# revision 22
# baseline (speedup 1.0000x reference)
"""Trainium2 Bass kernel for nn_DiscreteTimeS4.

Model (reference):
    x_proj = relu(x_seq @ W_in^T + b_in)                  # [B, T, P]
    h_t = a * h_{t-1} + x_proj_t @ B ;  y_t = h_t @ C     # diagonal SSM scan
    out = y @ W_out^T + b_out                             # [B, T, OUT]

Key transform: |a| <= sqrt(2/H) ~ 0.09, so a^k decays below fp32
precision within ~6 steps.  The scan is therefore (exactly, to fp32
precision) a short causal convolution over time, and W_out folds into
the conv matrices:
    out_t = sum_k x_proj_{t-k} @ F_k + b_out,
    F_k = B @ diag(a^k) @ C @ W_out^T          # [P, OUT], host-folded fp64
Device pipeline per 512-time chunk (all matmuls in float32r):
    stage 1: x_projT = relu(W_in @ x_T + b_in)            # PE + ACT
    stage 2: outT = sum_k F_k^T @ x_projT(shift k)        # K PSUM-accum mms,
             lag shifts are free SBUF column offsets      # -> [64, 512] PSUM
    out:     DVE 32x32 stream-transpose + strided DMA     # -> [512, 64] DRAM
b_out is added on the host (it is all-zero for this model's inputs).

Sharding: data-parallel over batch, 8 NeuronCores, B=64 -> 8 per core.
"""

import os
import sys

for _p in ("/opt/trn_rl_repo", "/root/.axon_site/_ro/trn_rl_repo"):
    if os.path.isdir(_p) and _p not in sys.path:
        sys.path.append(_p)

import numpy as np

import concourse.bacc as bacc
import concourse.mybir as mybir
from concourse.bass_utils import run_bass_kernel_spmd
from concourse.tile import TileContext

BATCH, T, IN, P, H, OUT = 64, 2048, 64, 128, 256, 64
NCORES = 8
BL = BATCH // NCORES          # batches per core
CHUNK = 512                   # time chunk (one fp32 PSUM bank)
NCHUNK = T // CHUNK

F32 = mybir.dt.float32
F32R = mybir.dt.float32r

_programs = {}                # (n_lags, reps) -> finalized Bacc program


def _build(n_lags: int, reps: int = 1):
    """Build the per-core Bass program for a fixed lag count.

    reps > 1 wraps the whole computation in an on-device loop executing
    it `reps` times — used only for benchmarking (amortizes the axon
    dispatch overhead, which dwarfs the kernel itself).
    """
    import contextlib

    nc = bacc.Bacc("TRN2", target_bir_lowering=False, num_devices=NCORES)

    x = nc.declare_dram_parameter("x", [BL, IN, T], F32, isOutput=False)
    wfold = nc.declare_dram_parameter("wfold", [n_lags, P, OUT], F32,
                                      isOutput=False)
    w_inT = nc.declare_dram_parameter("w_inT", [IN, P], F32, isOutput=False)
    b_in = nc.declare_dram_parameter("b_in", [P, 1], F32, isOutput=False)
    out = nc.declare_dram_parameter("out", [BL, T, OUT], F32, isOutput=True)

    PAD = n_lags - 1

    with TileContext(nc) as tc:
        with (
            tc.tile_pool(name="wpool", bufs=1) as wpool,
            tc.tile_pool(name="xin", bufs=2) as xin_pool,
            tc.tile_pool(name="xproj", bufs=2) as xp_pool,
            tc.tile_pool(name="btile", bufs=3) as bt_pool,
            tc.tile_pool(name="ps1", bufs=2, space="PSUM") as ps1_pool,
            tc.tile_pool(name="pso", bufs=3, space="PSUM") as pso_pool,
        ):
            # ---- load + round weights once ----
            fk32 = wpool.tile([P, n_lags * OUT], F32)
            for k in range(n_lags):
                nc.sync.dma_start(out=fk32[:, k * OUT:(k + 1) * OUT],
                                  in_=wfold[k])
            fk = wpool.tile([P, n_lags * OUT], F32R)
            nc.vector.tensor_copy(out=fk[:], in_=fk32[:])

            wi32 = wpool.tile([IN, P], F32)
            nc.sync.dma_start(out=wi32[:], in_=w_inT[:])
            wi = wpool.tile([IN, P], F32R)
            nc.vector.tensor_copy(out=wi[:], in_=wi32[:])

            bi = wpool.tile([P, 1], F32)
            nc.sync.dma_start(out=bi[:], in_=b_in[:])

            zpad = wpool.tile([P, PAD], F32)
            nc.vector.memset(zpad[:], 0.0)

            rep_ctx = (
                tc.For_i(
                    0, reps, 1,
                    hint_engines=(
                        mybir.EngineType.PE,
                        mybir.EngineType.DVE,
                        mybir.EngineType.Activation,
                        mybir.EngineType.SP,
                    ),
                )
                if reps > 1
                else contextlib.nullcontext()
            )
            with rep_ctx:
                _emit_body(nc, tc, n_lags, x, out, fk, wi, bi, zpad,
                           xin_pool, xp_pool, bt_pool, ps1_pool, pso_pool)

    nc.finalize()
    return nc


def _emit_body(nc, tc, n_lags, x, out, fk, wi, bi, zpad,
               xin_pool, xp_pool, bt_pool, ps1_pool, pso_pool):
    PAD = n_lags - 1
    for b in range(BL):
        # ---- load pre-transposed input [IN, T], round to f32r ----
        xT = xin_pool.tile([IN, T], F32, tag="xT")
        nc.sync.dma_start(out=xT[:], in_=x[b])
        xTr = xin_pool.tile([IN, T], F32R, tag="xTr")
        nc.vector.tensor_copy(out=xTr[:], in_=xT[:])

        # ---- stage 1: x_proj^T = relu(W_in @ xT + b_in) ----
        xp = xp_pool.tile([P, PAD + T], F32R)
        nc.vector.tensor_copy(out=xp[:, 0:PAD], in_=zpad[:])
        for c in range(NCHUNK):
            ps1 = ps1_pool.tile([P, CHUNK], F32)
            nc.tensor.matmul(
                ps1[:], wi[:], xTr[:, c * CHUNK:(c + 1) * CHUNK],
                start=True, stop=True,
            )
            # relu(ps1 + b_in) in one DVE op (ACT is ~4x slower/elem)
            nc.vector.tensor_scalar(
                out=xp[:, PAD + c * CHUNK: PAD + (c + 1) * CHUNK],
                in0=ps1[:],
                scalar1=bi[:],
                scalar2=0.0,
                op0=mybir.AluOpType.add,
                op1=mybir.AluOpType.max,
            )

        # ---- stage 2 (fused): outT[o, t] = sum_k F_k^T xp^T[:, t-k] ----
        for c in range(NCHUNK):
            pso = pso_pool.tile([OUT, CHUNK], F32)
            for k in range(n_lags):
                base = PAD + c * CHUNK - k
                nc.tensor.matmul(
                    pso[:], fk[:, k * OUT:(k + 1) * OUT],
                    xp[:, base: base + CHUNK],
                    start=(k == 0), stop=(k == n_lags - 1),
                )
            # ---- 32x32 block transpose + strided DMA -> [512, 64] ----
            bt = bt_pool.tile([OUT, CHUNK], F32)
            nc.vector.transpose(out=bt[:], in_=pso[:])
            for ob in range(OUT // 32):
                sb_view = bt[32 * ob:32 * (ob + 1), :].rearrange(
                    "ti (tb oi) -> ti tb oi", oi=32)
                d_view = out[b, c * CHUNK:(c + 1) * CHUNK,
                             32 * ob:32 * (ob + 1)].rearrange(
                    "(tb ti) oi -> ti tb oi", ti=32)
                nc.sync.dma_start(out=d_view, in_=sb_view)


def _n_lags(a: np.ndarray) -> int:
    amax = float(np.abs(a).max())
    if amax >= 1.0:
        return 16
    if amax <= 0.0:
        return 2
    k = int(np.ceil(np.log(2e-6) / np.log(amax)))
    return max(2, min(16, k))


def _prepare(x_seq, a, B, C, W_in, b_in, W_out, b_out):
    """Host-side folding + per-core input maps."""
    n_lags = _n_lags(a)
    a64 = a.astype(np.float64)
    B64 = B.astype(np.float64)
    C64 = C.astype(np.float64)
    CW64 = C64 @ W_out.T.astype(np.float64)                # [H, OUT]
    fks = np.stack(
        [(B64 * (a64 ** k)[None, :]) @ CW64 for k in range(n_lags)]
    ).astype(np.float32)                                   # [K, P, OUT]
    shared = {
        "wfold": np.ascontiguousarray(fks),
        "w_inT": np.ascontiguousarray(W_in.T.astype(np.float32)),
        "b_in": np.ascontiguousarray(b_in.astype(np.float32).reshape(P, 1)),
    }
    xT = np.ascontiguousarray(
        np.swapaxes(x_seq.astype(np.float32), 1, 2)
    )                                                      # [B, IN, T]
    in_maps = []
    for c in range(NCORES):
        m = dict(shared)
        m["x"] = xT[c * BL:(c + 1) * BL]
        in_maps.append(m)
    return n_lags, in_maps


def get_program(n_lags: int, reps: int = 1):
    key = (n_lags, reps)
    if key not in _programs:
        _programs[key] = _build(n_lags, reps)
    return _programs[key]


def kernel(x_seq, a, B, C, W_in, b_in, W_out, b_out):
    n_lags, in_maps = _prepare(x_seq, a, B, C, W_in, b_in, W_out, b_out)
    nc = get_program(n_lags)
    res = run_bass_kernel_spmd(nc, in_maps, list(range(NCORES)))
    out = np.concatenate([res.results[c]["out"] for c in range(NCORES)], axis=0)
    out = out.astype(np.float32)
    if np.any(b_out):
        out = out + b_out.astype(np.float32).reshape(1, 1, OUT)
    return out


# revision 24
# speedup vs baseline: 1.4634x; 1.4634x over previous
"""Trainium2 Bass kernel for nn_DiscreteTimeS4.

Model (reference):
    x_proj = relu(x_seq @ W_in^T + b_in)                  # [B, T, P]
    h_t = a * h_{t-1} + x_proj_t @ B ;  y_t = h_t @ C     # diagonal SSM scan
    out = y @ W_out^T + b_out                             # [B, T, OUT]

Key transform: |a| <= sqrt(2/H) ~ 0.09, so a^k decays below fp32
precision within a handful of steps.  The scan is therefore (exactly,
to fp32 precision) a short causal convolution over time, and W_out
folds into the conv matrices:
    out_t = sum_k x_proj_{t-k} @ F_k + b_out,
    F_k = B @ diag(a^k) @ C @ W_out^T          # [P, OUT], host-folded fp64
Device pipeline per batch row (all matmul operands fp16 — same 11-bit
mantissa as TF32/float32r, fp32 PSUM accumulation; end-to-end error
~4e-4 of output scale):
    stage 1: x_projT = relu(W_in @ x_T + b_in)     # PE mm + DVE bias-relu
    stage 2: outT = sum_k F_k^T @ x_projT(shift k) # K PSUM-accum mms; lag
             shifts are free SBUF column offsets; chunk PAIRS run
             concurrently in disjoint PE column groups (tile_position)
             -> [128, 512] PSUM = two [64, 512] chunk results
    out:     DVE 32x32 stream-transpose + strided DMA -> [512, 64] DRAM
b_out is added on the host (it is all-zero for this model's inputs).

Sharding: data-parallel over batch, 8 NeuronCores, B=64 -> 8 per core.
"""

import os
import sys

for _p in ("/opt/trn_rl_repo", "/root/.axon_site/_ro/trn_rl_repo"):
    if os.path.isdir(_p) and _p not in sys.path:
        sys.path.append(_p)

import numpy as np

import concourse.bacc as bacc
import concourse.mybir as mybir
from concourse.bass_utils import run_bass_kernel_spmd
from concourse.tile import TileContext

BATCH, T, IN, P, H, OUT = 64, 2048, 64, 128, 256, 64
NCORES = 8
BL = BATCH // NCORES          # batches per core
CHUNK = 512                   # time chunk (one fp32 PSUM bank)
NCHUNK = T // CHUNK

F32 = mybir.dt.float32
F16 = mybir.dt.float16

_programs = {}                # (n_lags, reps) -> finalized Bacc program


def _build(n_lags: int, reps: int = 1):
    """Build the per-core Bass program for a fixed lag count.

    reps > 1 wraps the whole computation in an on-device loop executing
    it `reps` times — used only for benchmarking (amortizes the axon
    dispatch overhead, which dwarfs the kernel itself).
    """
    import contextlib

    nc = bacc.Bacc("TRN2", target_bir_lowering=False, num_devices=NCORES)

    x = nc.declare_dram_parameter("x", [BL, IN, T], F16, isOutput=False)
    wfold = nc.declare_dram_parameter("wfold", [n_lags, P, OUT], F16,
                                      isOutput=False)
    w_inT = nc.declare_dram_parameter("w_inT", [IN, P], F16, isOutput=False)
    b_in = nc.declare_dram_parameter("b_in", [P, 1], F32, isOutput=False)
    out = nc.declare_dram_parameter("out", [BL, T, OUT], F32, isOutput=True)

    PAD = n_lags - 1

    with TileContext(nc) as tc:
        with (
            tc.tile_pool(name="wpool", bufs=1) as wpool,
            tc.tile_pool(name="xin", bufs=2) as xin_pool,
            tc.tile_pool(name="xproj", bufs=2) as xp_pool,
            tc.tile_pool(name="btile", bufs=3) as bt_pool,
            tc.tile_pool(name="ps1", bufs=2, space="PSUM") as ps1_pool,
            tc.tile_pool(name="pso", bufs=2, space="PSUM") as pso_pool,
        ):
            # ---- load weights once (already fp16 from host) ----
            fk = wpool.tile([P, n_lags * OUT], F16)
            for k in range(n_lags):
                nc.sync.dma_start(out=fk[:, k * OUT:(k + 1) * OUT],
                                  in_=wfold[k])
            wi = wpool.tile([IN, P], F16)
            nc.sync.dma_start(out=wi[:], in_=w_inT[:])
            bi = wpool.tile([P, 1], F32)
            nc.sync.dma_start(out=bi[:], in_=b_in[:])

            rep_ctx = (
                tc.For_i(
                    0, reps, 1,
                    hint_engines=(
                        mybir.EngineType.PE,
                        mybir.EngineType.DVE,
                        mybir.EngineType.Activation,
                        mybir.EngineType.SP,
                    ),
                )
                if reps > 1
                else contextlib.nullcontext()
            )
            with rep_ctx:
                _emit_body(nc, tc, n_lags, x, out, fk, wi, bi,
                           xin_pool, xp_pool, bt_pool, ps1_pool, pso_pool)

    nc.finalize()
    return nc


def _emit_body(nc, tc, n_lags, x, out, fk, wi, bi,
               xin_pool, xp_pool, bt_pool, ps1_pool, pso_pool):
    PAD = n_lags - 1
    for b in range(BL):
        # ---- load pre-transposed fp16 input [IN, T] ----
        xTr = xin_pool.tile([IN, T], F16, tag="xTr")
        nc.sync.dma_start(out=xTr[:], in_=x[b])

        # ---- stage 1: x_proj^T = relu(W_in @ xT + b_in) ----
        xp = xp_pool.tile([P, PAD + T], F16)
        nc.vector.memset(xp[:, 0:PAD], 0.0)
        for c in range(NCHUNK):
            ps1 = ps1_pool.tile([P, CHUNK], F32)
            nc.tensor.matmul(
                ps1[:], wi[:], xTr[:, c * CHUNK:(c + 1) * CHUNK],
                start=True, stop=True,
            )
            # relu(ps1 + b_in) in one DVE op (ACT is ~4x slower/elem)
            nc.vector.tensor_scalar(
                out=xp[:, PAD + c * CHUNK: PAD + (c + 1) * CHUNK],
                in0=ps1[:],
                scalar1=bi[:],
                scalar2=0.0,
                op0=mybir.AluOpType.add,
                op1=mybir.AluOpType.max,
            )

        # ---- stage 2 (fused): outT[o, t] = sum_k F_k^T xp^T[:, t-k] ----
        # Chunk pairs run concurrently in disjoint PE column groups
        # (tile_position), stacking two [64, CHUNK] results into one
        # [128, CHUNK] PSUM tile -> single transpose covers both.
        for c2 in range(NCHUNK // 2):
            pso = pso_pool.tile([2 * OUT, CHUNK], F32)
            for k in range(n_lags):
                for half in range(2):
                    c = 2 * c2 + half
                    base = PAD + c * CHUNK - k
                    nc.tensor.matmul(
                        pso[half * OUT:(half + 1) * OUT, :],
                        fk[:, k * OUT:(k + 1) * OUT],
                        xp[:, base: base + CHUNK],
                        start=(k == 0), stop=(k == n_lags - 1),
                        tile_position=(0, half * OUT),
                    )
            # ---- 32x32 block transpose + strided DMA -> [512, 64] x2 ----
            bt = bt_pool.tile([2 * OUT, CHUNK], F32)
            nc.vector.transpose(out=bt[:], in_=pso[:])
            for half in range(2):
                c = 2 * c2 + half
                for ob in range(OUT // 32):
                    p0 = half * OUT + 32 * ob
                    sb_view = bt[p0:p0 + 32, :].rearrange(
                        "ti (tb oi) -> ti tb oi", oi=32)
                    d_view = out[b, c * CHUNK:(c + 1) * CHUNK,
                                 32 * ob:32 * (ob + 1)].rearrange(
                        "(tb ti) oi -> ti tb oi", ti=32)
                    nc.sync.dma_start(out=d_view, in_=sb_view)


def _n_lags(a: np.ndarray) -> int:
    amax = float(np.abs(a).max())
    if amax >= 1.0:
        return 16
    if amax <= 0.0:
        return 2
    # fp16 operand noise floor is ~5e-4 of output scale; truncating the
    # tail at a^k < 5e-5 keeps truncation far below it.
    k = int(np.ceil(np.log(5e-5) / np.log(amax)))
    return max(2, min(16, k))


def _prepare(x_seq, a, B, C, W_in, b_in, W_out, b_out):
    """Host-side folding + per-core input maps."""
    n_lags = _n_lags(a)
    a64 = a.astype(np.float64)
    B64 = B.astype(np.float64)
    C64 = C.astype(np.float64)
    CW64 = C64 @ W_out.T.astype(np.float64)                # [H, OUT]
    fks = np.stack(
        [(B64 * (a64 ** k)[None, :]) @ CW64 for k in range(n_lags)]
    ).astype(np.float16)                                   # [K, P, OUT]
    shared = {
        "wfold": np.ascontiguousarray(fks),
        "w_inT": np.ascontiguousarray(W_in.T.astype(np.float16)),
        "b_in": np.ascontiguousarray(b_in.astype(np.float32).reshape(P, 1)),
    }
    xT = np.ascontiguousarray(
        np.swapaxes(x_seq, 1, 2).astype(np.float16)
    )                                                      # [B, IN, T]
    in_maps = []
    for c in range(NCORES):
        m = dict(shared)
        m["x"] = xT[c * BL:(c + 1) * BL]
        in_maps.append(m)
    return n_lags, in_maps


def get_program(n_lags: int, reps: int = 1):
    key = (n_lags, reps)
    if key not in _programs:
        _programs[key] = _build(n_lags, reps)
    return _programs[key]


def kernel(x_seq, a, B, C, W_in, b_in, W_out, b_out):
    n_lags, in_maps = _prepare(x_seq, a, B, C, W_in, b_in, W_out, b_out)
    nc = get_program(n_lags)
    res = run_bass_kernel_spmd(nc, in_maps, list(range(NCORES)))
    out = np.concatenate([res.results[c]["out"] for c in range(NCORES)], axis=0)
    out = out.astype(np.float32)
    if np.any(b_out):
        out = out + b_out.astype(np.float32).reshape(1, 1, OUT)
    return out


# revision 27
# speedup vs baseline: 1.5001x; 1.0250x over previous
"""Trainium2 Bass kernel for nn_DiscreteTimeS4.

Model (reference):
    x_proj = relu(x_seq @ W_in^T + b_in)                  # [B, T, P]
    h_t = a * h_{t-1} + x_proj_t @ B ;  y_t = h_t @ C     # diagonal SSM scan
    out = y @ W_out^T + b_out                             # [B, T, OUT]

Key transform: |a| <= sqrt(2/H) ~ 0.09, so a^k decays below fp32
precision within a handful of steps.  The scan is therefore (exactly,
to fp32 precision) a short causal convolution over time, and W_out
folds into the conv matrices:
    out_t = sum_k x_proj_{t-k} @ F_k + b_out,
    F_k = B @ diag(a^k) @ C @ W_out^T          # [P, OUT], host-folded fp64
Device pipeline per batch row (all matmul operands fp16 — same 11-bit
mantissa as TF32/float32r, fp32 PSUM accumulation; end-to-end error
~4e-4 of output scale):
    stage 1: x_projT = relu(W_in @ x_T + b_in)     # PE mm + DVE bias-relu
    stage 2: outT = sum_k F_k^T @ x_projT(shift k) # K PSUM-accum mms; lag
             shifts are free SBUF column offsets; chunk PAIRS run
             concurrently in disjoint PE column groups (tile_position)
             -> [128, 512] PSUM = two [64, 512] chunk results
    out:     DVE 32x32 stream-transpose + strided DMA -> [512, 64] DRAM
b_out is added on the host (it is all-zero for this model's inputs).

Sharding: data-parallel over batch, 8 NeuronCores, B=64 -> 8 per core.
"""

import os
import sys

for _p in ("/opt/trn_rl_repo", "/root/.axon_site/_ro/trn_rl_repo"):
    if os.path.isdir(_p) and _p not in sys.path:
        sys.path.append(_p)

import numpy as np

import concourse.bacc as bacc
import concourse.mybir as mybir
from concourse.bass_utils import run_bass_kernel_spmd
from concourse.tile import TileContext

BATCH, T, IN, P, H, OUT = 64, 2048, 64, 128, 256, 64
NCORES = 8
BL = BATCH // NCORES          # batches per core
CHUNK = 512                   # time chunk (one fp32 PSUM bank)
NCHUNK = T // CHUNK

F32 = mybir.dt.float32
F16 = mybir.dt.float16

_programs = {}                # (n_lags, reps) -> finalized Bacc program


def _build(n_lags: int, reps: int = 1):
    """Build the per-core Bass program for a fixed lag count.

    reps > 1 wraps the whole computation in an on-device loop executing
    it `reps` times — used only for benchmarking (amortizes the axon
    dispatch overhead, which dwarfs the kernel itself).
    """
    import contextlib

    nc = bacc.Bacc("TRN2", target_bir_lowering=False, num_devices=NCORES)

    x = nc.declare_dram_parameter("x", [BL, IN, T], F16, isOutput=False)
    wfold = nc.declare_dram_parameter("wfold", [n_lags, P, OUT], F16,
                                      isOutput=False)
    w_inT = nc.declare_dram_parameter("w_inT", [IN, P], F16, isOutput=False)
    b_in = nc.declare_dram_parameter("b_in", [P, 1], F32, isOutput=False)
    out = nc.declare_dram_parameter("out", [BL, T, OUT], F32, isOutput=True)

    PAD = n_lags - 1

    with TileContext(nc) as tc:
        with (
            tc.tile_pool(name="wpool", bufs=1) as wpool,
            tc.tile_pool(name="xin", bufs=2) as xin_pool,
            tc.tile_pool(name="xproj", bufs=2) as xp_pool,
            tc.tile_pool(name="btile", bufs=3) as bt_pool,
            tc.tile_pool(name="ps1", bufs=4, space="PSUM") as ps1_pool,
            tc.tile_pool(name="pso", bufs=3, space="PSUM") as pso_pool,
        ):
            # ---- load weights once (already fp16 from host) ----
            fk = wpool.tile([P, n_lags * OUT], F16)
            for k in range(n_lags):
                nc.sync.dma_start(out=fk[:, k * OUT:(k + 1) * OUT],
                                  in_=wfold[k])
            wi = wpool.tile([IN, P], F16)
            nc.sync.dma_start(out=wi[:], in_=w_inT[:])
            bi = wpool.tile([P, 1], F32)
            nc.sync.dma_start(out=bi[:], in_=b_in[:])

            rep_ctx = (
                tc.For_i(
                    0, reps, 1,
                    hint_engines=(
                        mybir.EngineType.PE,
                        mybir.EngineType.DVE,
                        mybir.EngineType.Activation,
                        mybir.EngineType.SP,
                    ),
                )
                if reps > 1
                else contextlib.nullcontext()
            )
            with rep_ctx:
                _emit_body(nc, tc, n_lags, x, out, fk, wi, bi,
                           xin_pool, xp_pool, bt_pool, ps1_pool, pso_pool)

    nc.finalize()
    return nc


def _emit_body(nc, tc, n_lags, x, out, fk, wi, bi,
               xin_pool, xp_pool, bt_pool, ps1_pool, pso_pool):
    PAD = n_lags - 1
    for b in range(BL):
        # ---- load pre-transposed fp16 input [IN, T] ----
        xTr = xin_pool.tile([IN, T], F16, tag="xTr")
        nc.sync.dma_start(out=xTr[:], in_=x[b])

        # ---- stage 1: x_proj^T = relu(W_in @ xT + b_in) ----
        xp = xp_pool.tile([P, PAD + T], F16)
        nc.vector.memset(xp[:, 0:PAD], 0.0)
        for c in range(NCHUNK):
            ps1 = ps1_pool.tile([P, CHUNK], F32)
            nc.tensor.matmul(
                ps1[:], wi[:], xTr[:, c * CHUNK:(c + 1) * CHUNK],
                start=True, stop=True,
            )
            # relu(ps1 + b_in); mostly on DVE (fast), 1-in-4 on the
            # otherwise-idle ACT engine to share the load
            if c == NCHUNK - 1:
                nc.scalar.activation(
                    out=xp[:, PAD + c * CHUNK: PAD + (c + 1) * CHUNK],
                    in_=ps1[:],
                    func=mybir.ActivationFunctionType.Relu,
                    bias=bi[:],
                )
            else:
                nc.vector.tensor_scalar(
                    out=xp[:, PAD + c * CHUNK: PAD + (c + 1) * CHUNK],
                    in0=ps1[:],
                    scalar1=bi[:],
                    scalar2=0.0,
                    op0=mybir.AluOpType.add,
                    op1=mybir.AluOpType.max,
                )

        # ---- stage 2 (fused): outT[o, t] = sum_k F_k^T xp^T[:, t-k] ----
        # Chunk pairs run concurrently in disjoint PE column groups
        # (tile_position), stacking two [64, CHUNK] results into one
        # [128, CHUNK] PSUM tile -> single transpose covers both.
        for c2 in range(NCHUNK // 2):
            pso = pso_pool.tile([2 * OUT, CHUNK], F32)
            for k in range(n_lags):
                for half in range(2):
                    c = 2 * c2 + half
                    base = PAD + c * CHUNK - k
                    nc.tensor.matmul(
                        pso[half * OUT:(half + 1) * OUT, :],
                        fk[:, k * OUT:(k + 1) * OUT],
                        xp[:, base: base + CHUNK],
                        start=(k == 0), stop=(k == n_lags - 1),
                        tile_position=(0, half * OUT),
                    )
            # ---- 32x32 block transpose + strided DMA -> [512, 64] x2 ----
            bt = bt_pool.tile([2 * OUT, CHUNK], F32)
            nc.vector.transpose(out=bt[:], in_=pso[:])
            for half in range(2):
                c = 2 * c2 + half
                for ob in range(OUT // 32):
                    p0 = half * OUT + 32 * ob
                    sb_view = bt[p0:p0 + 32, :].rearrange(
                        "ti (tb oi) -> ti tb oi", oi=32)
                    d_view = out[b, c * CHUNK:(c + 1) * CHUNK,
                                 32 * ob:32 * (ob + 1)].rearrange(
                        "(tb ti) oi -> ti tb oi", ti=32)
                    # issue on the ACT HWDGE ring; input DMAs use SP's
                    nc.scalar.dma_start(out=d_view, in_=sb_view)


def _n_lags(a: np.ndarray) -> int:
    amax = float(np.abs(a).max())
    if amax >= 1.0:
        return 16
    if amax <= 0.0:
        return 2
    # fp16 operand noise floor is ~5e-4 of output scale; truncating the
    # tail at a^k < 5e-5 keeps truncation far below it.
    k = int(np.ceil(np.log(5e-5) / np.log(amax)))
    return max(2, min(16, k))


def _prepare(x_seq, a, B, C, W_in, b_in, W_out, b_out):
    """Host-side folding + per-core input maps."""
    n_lags = _n_lags(a)
    a64 = a.astype(np.float64)
    B64 = B.astype(np.float64)
    C64 = C.astype(np.float64)
    CW64 = C64 @ W_out.T.astype(np.float64)                # [H, OUT]
    fks = np.stack(
        [(B64 * (a64 ** k)[None, :]) @ CW64 for k in range(n_lags)]
    ).astype(np.float16)                                   # [K, P, OUT]
    shared = {
        "wfold": np.ascontiguousarray(fks),
        "w_inT": np.ascontiguousarray(W_in.T.astype(np.float16)),
        "b_in": np.ascontiguousarray(b_in.astype(np.float32).reshape(P, 1)),
    }
    xT = np.ascontiguousarray(
        np.swapaxes(x_seq, 1, 2).astype(np.float16)
    )                                                      # [B, IN, T]
    in_maps = []
    for c in range(NCORES):
        m = dict(shared)
        m["x"] = xT[c * BL:(c + 1) * BL]
        in_maps.append(m)
    return n_lags, in_maps


def get_program(n_lags: int, reps: int = 1):
    key = (n_lags, reps)
    if key not in _programs:
        _programs[key] = _build(n_lags, reps)
    return _programs[key]


def kernel(x_seq, a, B, C, W_in, b_in, W_out, b_out):
    n_lags, in_maps = _prepare(x_seq, a, B, C, W_in, b_in, W_out, b_out)
    nc = get_program(n_lags)
    res = run_bass_kernel_spmd(nc, in_maps, list(range(NCORES)))
    out = np.concatenate([res.results[c]["out"] for c in range(NCORES)], axis=0)
    out = out.astype(np.float32)
    if np.any(b_out):
        out = out + b_out.astype(np.float32).reshape(1, 1, OUT)
    return out


# revision 28
# speedup vs baseline: 1.5696x; 1.0464x over previous
"""Trainium2 Bass kernel for nn_DiscreteTimeS4.

Model (reference):
    x_proj = relu(x_seq @ W_in^T + b_in)                  # [B, T, P]
    h_t = a * h_{t-1} + x_proj_t @ B ;  y_t = h_t @ C     # diagonal SSM scan
    out = y @ W_out^T + b_out                             # [B, T, OUT]

Key transform: |a| <= sqrt(2/H) ~ 0.09, so a^k decays below fp32
precision within a handful of steps.  The scan is therefore (exactly,
to fp32 precision) a short causal convolution over time, and W_out
folds into the conv matrices:
    out_t = sum_k x_proj_{t-k} @ F_k + b_out,
    F_k = B @ diag(a^k) @ C @ W_out^T          # [P, OUT], host-folded fp64
Device pipeline per batch row (all matmul operands fp16 — same 11-bit
mantissa as TF32/float32r, fp32 PSUM accumulation; end-to-end error
~4e-4 of output scale):
    stage 1: x_projT = relu(W_in @ x_T + b_in)     # PE mm + DVE bias-relu
    stage 2: outT = sum_k F_k^T @ x_projT(shift k) # K PSUM-accum mms; lag
             shifts are free SBUF column offsets; chunk PAIRS run
             concurrently in disjoint PE column groups (tile_position)
             -> [128, 512] PSUM = two [64, 512] chunk results
    out:     DVE 32x32 stream-transpose + strided DMA -> [512, 64] DRAM
b_out is added on the host (it is all-zero for this model's inputs).

Sharding: data-parallel over batch, 8 NeuronCores, B=64 -> 8 per core.
"""

import os
import sys

for _p in ("/opt/trn_rl_repo", "/root/.axon_site/_ro/trn_rl_repo"):
    if os.path.isdir(_p) and _p not in sys.path:
        sys.path.append(_p)

import numpy as np

import concourse.bacc as bacc
import concourse.mybir as mybir
from concourse.bass_utils import run_bass_kernel_spmd
from concourse.tile import TileContext

BATCH, T, IN, P, H, OUT = 64, 2048, 64, 128, 256, 64
NCORES = 8
BL = BATCH // NCORES          # batches per core
CHUNK = 512                   # time chunk (one fp32 PSUM bank)
NCHUNK = T // CHUNK

F32 = mybir.dt.float32
F16 = mybir.dt.float16

_programs = {}                # (n_lags, reps) -> finalized Bacc program


def _build(n_lags: int, reps: int = 1):
    """Build the per-core Bass program for a fixed lag count.

    reps > 1 wraps the whole computation in an on-device loop executing
    it `reps` times — used only for benchmarking (amortizes the axon
    dispatch overhead, which dwarfs the kernel itself).
    """
    import contextlib

    nc = bacc.Bacc("TRN2", target_bir_lowering=False, num_devices=NCORES)

    x = nc.declare_dram_parameter("x", [BL, IN, T], F16, isOutput=False)
    wfold = nc.declare_dram_parameter("wfold", [n_lags, P, OUT], F16,
                                      isOutput=False)
    w_inT = nc.declare_dram_parameter("w_inT", [IN, P], F16, isOutput=False)
    b_in = nc.declare_dram_parameter("b_in", [P, 1], F32, isOutput=False)
    out = nc.declare_dram_parameter("out", [BL, T, OUT], F32, isOutput=True)

    PAD = n_lags - 1

    with TileContext(nc) as tc:
        with (
            tc.tile_pool(name="wpool", bufs=1) as wpool,
            tc.tile_pool(name="xin", bufs=2) as xin_pool,
            tc.tile_pool(name="xproj", bufs=2) as xp_pool,
            tc.tile_pool(name="btile", bufs=3) as bt_pool,
            tc.tile_pool(name="ps1", bufs=4, space="PSUM") as ps1_pool,
            tc.tile_pool(name="pso", bufs=3, space="PSUM") as pso_pool,
        ):
            # ---- load weights once (already fp16 from host) ----
            fk = wpool.tile([P, n_lags * OUT], F16)
            for k in range(n_lags):
                nc.sync.dma_start(out=fk[:, k * OUT:(k + 1) * OUT],
                                  in_=wfold[k])
            wi = wpool.tile([IN, P], F16)
            nc.sync.dma_start(out=wi[:], in_=w_inT[:])
            bi = wpool.tile([P, 1], F32)
            nc.sync.dma_start(out=bi[:], in_=b_in[:])

            rep_ctx = (
                tc.For_i(
                    0, reps, 1,
                    hint_engines=(
                        mybir.EngineType.PE,
                        mybir.EngineType.DVE,
                        mybir.EngineType.Activation,
                        mybir.EngineType.SP,
                    ),
                )
                if reps > 1
                else contextlib.nullcontext()
            )
            with rep_ctx:
                _emit_body(nc, tc, n_lags, x, out, fk, wi, bi,
                           xin_pool, xp_pool, bt_pool, ps1_pool, pso_pool)

    nc.finalize()
    return nc


def _emit_body(nc, tc, n_lags, x, out, fk, wi, bi,
               xin_pool, xp_pool, bt_pool, ps1_pool, pso_pool):
    PAD = n_lags - 1

    def stage1(b):
        # ---- load pre-transposed fp16 input [IN, T] ----
        xTr = xin_pool.tile([IN, T], F16, tag="xTr")
        nc.sync.dma_start(out=xTr[:], in_=x[b])

        # ---- stage 1: x_proj^T = relu(W_in @ xT + b_in) ----
        xp = xp_pool.tile([P, PAD + T], F16, tag="xp")
        nc.vector.memset(xp[:, 0:PAD], 0.0)
        for c in range(NCHUNK):
            ps1 = ps1_pool.tile([P, CHUNK], F32)
            nc.tensor.matmul(
                ps1[:], wi[:], xTr[:, c * CHUNK:(c + 1) * CHUNK],
                start=True, stop=True,
            )
            # relu(ps1 + b_in); mostly on DVE (fast), 1-in-4 on the
            # otherwise-idle ACT engine to share the load
            if c == NCHUNK - 1:
                nc.scalar.activation(
                    out=xp[:, PAD + c * CHUNK: PAD + (c + 1) * CHUNK],
                    in_=ps1[:],
                    func=mybir.ActivationFunctionType.Relu,
                    bias=bi[:],
                )
            else:
                nc.vector.tensor_scalar(
                    out=xp[:, PAD + c * CHUNK: PAD + (c + 1) * CHUNK],
                    in0=ps1[:],
                    scalar1=bi[:],
                    scalar2=0.0,
                    op0=mybir.AluOpType.add,
                    op1=mybir.AluOpType.max,
                )
        return xp

    def stage2(b, xp):
        # ---- stage 2 (fused): outT[o, t] = sum_k F_k^T xp^T[:, t-k] ----
        # Chunk pairs run concurrently in disjoint PE column groups
        # (tile_position), stacking two [64, CHUNK] results into one
        # [128, CHUNK] PSUM tile -> single transpose covers both.
        for c2 in range(NCHUNK // 2):
            pso = pso_pool.tile([2 * OUT, CHUNK], F32)
            for k in range(n_lags):
                for half in range(2):
                    c = 2 * c2 + half
                    base = PAD + c * CHUNK - k
                    nc.tensor.matmul(
                        pso[half * OUT:(half + 1) * OUT, :],
                        fk[:, k * OUT:(k + 1) * OUT],
                        xp[:, base: base + CHUNK],
                        start=(k == 0), stop=(k == n_lags - 1),
                        tile_position=(0, half * OUT),
                    )
            # ---- 32x32 block transpose + strided DMA -> [512, 64] x2 ----
            bt = bt_pool.tile([2 * OUT, CHUNK], F32)
            nc.vector.transpose(out=bt[:], in_=pso[:])
            for half in range(2):
                c = 2 * c2 + half
                for ob in range(OUT // 32):
                    p0 = half * OUT + 32 * ob
                    sb_view = bt[p0:p0 + 32, :].rearrange(
                        "ti (tb oi) -> ti tb oi", oi=32)
                    d_view = out[b, c * CHUNK:(c + 1) * CHUNK,
                                 32 * ob:32 * (ob + 1)].rearrange(
                        "(tb ti) oi -> ti tb oi", ti=32)
                    # issue on the ACT HWDGE ring; input DMAs use SP's
                    nc.scalar.dma_start(out=d_view, in_=sb_view)

    # Software pipeline: emit stage1(b+1) before stage2(b) so the PE's
    # in-order stream never stalls on the relu chain of the same b.
    prev = None
    for b in range(BL):
        xp = stage1(b)
        if prev is not None:
            stage2(b - 1, prev)
        prev = xp
    stage2(BL - 1, prev)


def _n_lags(a: np.ndarray) -> int:
    amax = float(np.abs(a).max())
    if amax >= 1.0:
        return 16
    if amax <= 0.0:
        return 2
    # fp16 operand noise floor is ~5e-4 of output scale; truncating the
    # tail at a^k < 5e-5 keeps truncation far below it.
    k = int(np.ceil(np.log(5e-5) / np.log(amax)))
    return max(2, min(16, k))


def _prepare(x_seq, a, B, C, W_in, b_in, W_out, b_out):
    """Host-side folding + per-core input maps."""
    n_lags = _n_lags(a)
    a64 = a.astype(np.float64)
    B64 = B.astype(np.float64)
    C64 = C.astype(np.float64)
    CW64 = C64 @ W_out.T.astype(np.float64)                # [H, OUT]
    fks = np.stack(
        [(B64 * (a64 ** k)[None, :]) @ CW64 for k in range(n_lags)]
    ).astype(np.float16)                                   # [K, P, OUT]
    shared = {
        "wfold": np.ascontiguousarray(fks),
        "w_inT": np.ascontiguousarray(W_in.T.astype(np.float16)),
        "b_in": np.ascontiguousarray(b_in.astype(np.float32).reshape(P, 1)),
    }
    xT = np.ascontiguousarray(
        np.swapaxes(x_seq, 1, 2).astype(np.float16)
    )                                                      # [B, IN, T]
    in_maps = []
    for c in range(NCORES):
        m = dict(shared)
        m["x"] = xT[c * BL:(c + 1) * BL]
        in_maps.append(m)
    return n_lags, in_maps


def get_program(n_lags: int, reps: int = 1):
    key = (n_lags, reps)
    if key not in _programs:
        _programs[key] = _build(n_lags, reps)
    return _programs[key]


def kernel(x_seq, a, B, C, W_in, b_in, W_out, b_out):
    n_lags, in_maps = _prepare(x_seq, a, B, C, W_in, b_in, W_out, b_out)
    nc = get_program(n_lags)
    res = run_bass_kernel_spmd(nc, in_maps, list(range(NCORES)))
    out = np.concatenate([res.results[c]["out"] for c in range(NCORES)], axis=0)
    out = out.astype(np.float32)
    if np.any(b_out):
        out = out + b_out.astype(np.float32).reshape(1, 1, OUT)
    return out


# revision 30
# speedup vs baseline: 1.7236x; 1.0981x over previous
"""Trainium2 Bass kernel for nn_DiscreteTimeS4.

Model (reference):
    x_proj = relu(x_seq @ W_in^T + b_in)                  # [B, T, P]
    h_t = a * h_{t-1} + x_proj_t @ B ;  y_t = h_t @ C     # diagonal SSM scan
    out = y @ W_out^T + b_out                             # [B, T, OUT]

Key transform: |a| <= sqrt(2/H) ~ 0.09, so a^k decays below fp32
precision within a handful of steps.  The scan is therefore (exactly,
to fp32 precision) a short causal convolution over time, and W_out
folds into the conv matrices:
    out_t = sum_k x_proj_{t-k} @ F_k + b_out,
    F_k = B @ diag(a^k) @ C @ W_out^T          # [P, OUT], host-folded fp64
Device pipeline per batch row (all matmul operands fp16 — same 11-bit
mantissa as TF32/float32r, fp32 PSUM accumulation; end-to-end error
~4e-4 of output scale):
    stage 1: x_projT = relu(W_in @ x_T + b_in)     # PE mm + DVE bias-relu
    stage 2: outT = sum_k F_k^T @ x_projT(shift k) # K PSUM-accum mms; lag
             shifts are free SBUF column offsets; chunk PAIRS run
             concurrently in disjoint PE column groups (tile_position)
             -> [128, 512] PSUM = two [64, 512] chunk results
    out:     DVE 32x32 stream-transpose + strided DMA -> [512, 64] DRAM
b_out is added on the host (it is all-zero for this model's inputs).

Sharding: data-parallel over batch, 8 NeuronCores, B=64 -> 8 per core.
"""

import os
import sys

for _p in ("/opt/trn_rl_repo", "/root/.axon_site/_ro/trn_rl_repo"):
    if os.path.isdir(_p) and _p not in sys.path:
        sys.path.append(_p)

import numpy as np

import concourse.bacc as bacc
import concourse.mybir as mybir
from concourse.bass_utils import run_bass_kernel_spmd
from concourse.tile import TileContext

BATCH, T, IN, P, H, OUT = 64, 2048, 64, 128, 256, 64
NCORES = 8
BL = BATCH // NCORES          # batches per core
CHUNK = 512                   # time chunk (one fp32 PSUM bank)
NCHUNK = T // CHUNK

F32 = mybir.dt.float32
F16 = mybir.dt.float16

_programs = {}                # (n_lags, reps) -> finalized Bacc program


def _build(n_lags: int, reps: int = 1):
    """Build the per-core Bass program for a fixed lag count.

    reps > 1 wraps the whole computation in an on-device loop executing
    it `reps` times — used only for benchmarking (amortizes the axon
    dispatch overhead, which dwarfs the kernel itself).
    """
    import contextlib

    nc = bacc.Bacc("TRN2", target_bir_lowering=False, num_devices=NCORES)

    x = nc.declare_dram_parameter("x", [BL, IN, T], F16, isOutput=False)
    wfold = nc.declare_dram_parameter("wfold", [n_lags, P, OUT], F16,
                                      isOutput=False)
    w_inT = nc.declare_dram_parameter("w_inT", [IN, P], F16, isOutput=False)
    b_in = nc.declare_dram_parameter("b_in", [P, 1], F32, isOutput=False)
    out = nc.declare_dram_parameter("out", [BL, T, OUT], F32, isOutput=True)

    PAD = n_lags - 1

    with TileContext(nc) as tc:
        with (
            tc.tile_pool(name="wpool", bufs=1) as wpool,
            tc.tile_pool(name="xin", bufs=2) as xin_pool,
            tc.tile_pool(name="xproj", bufs=5) as xp_pool,
            tc.tile_pool(name="btile", bufs=3) as bt_pool,
            tc.tile_pool(name="ps1", bufs=4, space="PSUM") as ps1_pool,
            tc.tile_pool(name="pso", bufs=3, space="PSUM") as pso_pool,
        ):
            # ---- load weights once (already fp16 from host) ----
            fk = wpool.tile([P, n_lags * OUT], F16)
            for k in range(n_lags):
                nc.sync.dma_start(out=fk[:, k * OUT:(k + 1) * OUT],
                                  in_=wfold[k])
            wi = wpool.tile([IN, P], F16)
            nc.sync.dma_start(out=wi[:], in_=w_inT[:])
            bi = wpool.tile([P, 1], F32)
            nc.sync.dma_start(out=bi[:], in_=b_in[:])

            rep_ctx = (
                tc.For_i(
                    0, reps, 1,
                    hint_engines=(
                        mybir.EngineType.PE,
                        mybir.EngineType.DVE,
                        mybir.EngineType.Activation,
                        mybir.EngineType.SP,
                    ),
                )
                if reps > 1
                else contextlib.nullcontext()
            )
            with rep_ctx:
                _emit_body(nc, tc, n_lags, x, out, fk, wi, bi,
                           xin_pool, xp_pool, bt_pool, ps1_pool, pso_pool)

    nc.finalize()
    return nc


def _emit_body(nc, tc, n_lags, x, out, fk, wi, bi,
               xin_pool, xp_pool, bt_pool, ps1_pool, pso_pool):
    PAD = n_lags - 1
    NP2 = NCHUNK // 2       # chunk pairs per batch row
    HALF = 2 * CHUNK        # columns per pair

    def load_x(b):
        xTr = xin_pool.tile([IN, T], F16, tag="xTr")
        nc.sync.dma_start(out=xTr[:], in_=x[b])
        return xTr

    def stage1_pair(b, p, xTr, xp_prev):
        """stage 1 for chunks (2p, 2p+1) into a dedicated pair tile
        [P, PAD + 2*CHUNK]; pad head = zeros (p==0) or tail of the
        previous pair (copied)."""
        xp = xp_pool.tile([P, PAD + HALF], F16, tag="xpp")
        if p == 0:
            nc.vector.memset(xp[:, 0:PAD], 0.0)
        else:
            nc.vector.tensor_copy(out=xp[:, 0:PAD],
                                  in_=xp_prev[:, HALF:PAD + HALF])
        for h in range(2):
            c = 2 * p + h
            ps1 = ps1_pool.tile([P, CHUNK], F32)
            nc.tensor.matmul(
                ps1[:], wi[:], xTr[:, c * CHUNK:(c + 1) * CHUNK],
                start=True, stop=True,
            )
            # relu(ps1 + b_in): alternate DVE / ACT to split the load
            dst = xp[:, PAD + h * CHUNK: PAD + (h + 1) * CHUNK]
            if h == 1:
                nc.scalar.activation(
                    out=dst, in_=ps1[:],
                    func=mybir.ActivationFunctionType.Relu, bias=bi[:],
                )
            else:
                nc.vector.tensor_scalar(
                    out=dst, in0=ps1[:], scalar1=bi[:], scalar2=0.0,
                    op0=mybir.AluOpType.add, op1=mybir.AluOpType.max,
                )
        return xp

    def stage2_pair(b, p, xp):
        """fused conv for chunk pair p: two chunks concurrently in
        disjoint PE column groups -> [128, CHUNK] PSUM -> transpose ->
        strided DMA."""
        pso = pso_pool.tile([2 * OUT, CHUNK], F32)
        for k in range(n_lags):
            for half in range(2):
                base = PAD + half * CHUNK - k
                nc.tensor.matmul(
                    pso[half * OUT:(half + 1) * OUT, :],
                    fk[:, k * OUT:(k + 1) * OUT],
                    xp[:, base: base + CHUNK],
                    start=(k == 0), stop=(k == n_lags - 1),
                    tile_position=(0, half * OUT),
                )
        bt = bt_pool.tile([2 * OUT, CHUNK], F32)
        nc.vector.transpose(out=bt[:], in_=pso[:])
        for half in range(2):
            c = 2 * p + half
            for ob in range(OUT // 32):
                p0 = half * OUT + 32 * ob
                sb_view = bt[p0:p0 + 32, :].rearrange(
                    "ti (tb oi) -> ti tb oi", oi=32)
                d_view = out[b, c * CHUNK:(c + 1) * CHUNK,
                             32 * ob:32 * (ob + 1)].rearrange(
                    "(tb ti) oi -> ti tb oi", ti=32)
                # issue on the ACT HWDGE ring; input DMAs use SP's
                nc.scalar.dma_start(out=d_view, in_=sb_view)

    # Chunk-pair-level software pipeline, depth 2: stage2(i) is emitted
    # after stage1(i+2), so each stage2's relu inputs have two full
    # stage-1 windows plus a stage2 of PE time to land.
    DEPTH = 2
    work = [(b, p) for b in range(BL) for p in range(NP2)]
    s1_done = {}
    xTr_cur = None
    xp_prev = None
    for i, (b, p) in enumerate(work):
        if p == 0:
            xTr_cur = load_x(b)
            xp_prev = None
        xp_prev = stage1_pair(b, p, xTr_cur, xp_prev)
        s1_done[i] = (b, p, xp_prev)
        j = i - DEPTH
        if j >= 0:
            bb, pp, xpp = s1_done.pop(j)
            stage2_pair(bb, pp, xpp)
    for j in sorted(s1_done):
        bb, pp, xpp = s1_done.pop(j)
        stage2_pair(bb, pp, xpp)


def _n_lags(a: np.ndarray) -> int:
    amax = float(np.abs(a).max())
    if amax >= 1.0:
        return 16
    if amax <= 0.0:
        return 2
    # fp16 operand noise floor is ~5e-4 of output scale; truncating the
    # tail at a^k < 5e-5 keeps truncation far below it.
    k = int(np.ceil(np.log(5e-5) / np.log(amax)))
    return max(2, min(16, k))


def _prepare(x_seq, a, B, C, W_in, b_in, W_out, b_out):
    """Host-side folding + per-core input maps."""
    n_lags = _n_lags(a)
    a64 = a.astype(np.float64)
    B64 = B.astype(np.float64)
    C64 = C.astype(np.float64)
    CW64 = C64 @ W_out.T.astype(np.float64)                # [H, OUT]
    fks = np.stack(
        [(B64 * (a64 ** k)[None, :]) @ CW64 for k in range(n_lags)]
    ).astype(np.float16)                                   # [K, P, OUT]
    shared = {
        "wfold": np.ascontiguousarray(fks),
        "w_inT": np.ascontiguousarray(W_in.T.astype(np.float16)),
        "b_in": np.ascontiguousarray(b_in.astype(np.float32).reshape(P, 1)),
    }
    xT = np.ascontiguousarray(
        np.swapaxes(x_seq, 1, 2).astype(np.float16)
    )                                                      # [B, IN, T]
    in_maps = []
    for c in range(NCORES):
        m = dict(shared)
        m["x"] = xT[c * BL:(c + 1) * BL]
        in_maps.append(m)
    return n_lags, in_maps


def get_program(n_lags: int, reps: int = 1):
    key = (n_lags, reps)
    if key not in _programs:
        _programs[key] = _build(n_lags, reps)
    return _programs[key]


def kernel(x_seq, a, B, C, W_in, b_in, W_out, b_out):
    n_lags, in_maps = _prepare(x_seq, a, B, C, W_in, b_in, W_out, b_out)
    nc = get_program(n_lags)
    res = run_bass_kernel_spmd(nc, in_maps, list(range(NCORES)))
    out = np.concatenate([res.results[c]["out"] for c in range(NCORES)], axis=0)
    out = out.astype(np.float32)
    if np.any(b_out):
        out = out + b_out.astype(np.float32).reshape(1, 1, OUT)
    return out


# revision 33
# speedup vs baseline: 1.9735x; 1.1450x over previous
"""Trainium2 Bass kernel for nn_DiscreteTimeS4.

Model (reference):
    x_proj = relu(x_seq @ W_in^T + b_in)                  # [B, T, P]
    h_t = a * h_{t-1} + x_proj_t @ B ;  y_t = h_t @ C     # diagonal SSM scan
    out = y @ W_out^T + b_out                             # [B, T, OUT]

Key transform: |a| <= sqrt(2/H) ~ 0.09, so a^k decays below fp32
precision within a handful of steps.  The scan is therefore (exactly,
to fp32 precision) a short causal convolution over time, and W_out
folds into the conv matrices:
    out_t = sum_k x_proj_{t-k} @ F_k + b_out,
    F_k = B @ diag(a^k) @ C @ W_out^T          # [P, OUT], host-folded fp64
Device pipeline per batch row (all matmul operands fp16 — same 11-bit
mantissa as TF32/float32r, fp32 PSUM accumulation; end-to-end error
~4e-4 of output scale):
    stage 1: x_projT = relu(W_in @ x_T + b_in)     # PE mm + DVE bias-relu
    stage 2: outT = sum_k F_k^T @ x_projT(shift k) # K PSUM-accum mms; lag
             shifts are free SBUF column offsets; chunk PAIRS run
             concurrently in disjoint PE column groups (tile_position)
             -> [128, 512] PSUM = two [64, 512] chunk results
    out:     DVE 32x32 stream-transpose + strided DMA -> [512, 64] DRAM
b_out is added on the host (it is all-zero for this model's inputs).

Sharding: data-parallel over batch, 8 NeuronCores, B=64 -> 8 per core.
"""

import os
import sys

for _p in ("/opt/trn_rl_repo", "/root/.axon_site/_ro/trn_rl_repo"):
    if os.path.isdir(_p) and _p not in sys.path:
        sys.path.append(_p)

import numpy as np

import concourse.bacc as bacc
import concourse.mybir as mybir
from concourse.bass_utils import run_bass_kernel_spmd
from concourse.tile import TileContext

BATCH, T, IN, P, H, OUT = 64, 2048, 64, 128, 256, 64
NCORES = 8
BL = BATCH // NCORES          # batches per core
CHUNK = 512                   # time chunk (one fp32 PSUM bank)
NCHUNK = T // CHUNK

F32 = mybir.dt.float32
F16 = mybir.dt.float16

_programs = {}                # (n_lags, reps) -> finalized Bacc program


def _build(n_lags: int, reps: int = 1):
    """Build the per-core Bass program for a fixed lag count.

    reps > 1 wraps the whole computation in an on-device loop executing
    it `reps` times — used only for benchmarking (amortizes the axon
    dispatch overhead, which dwarfs the kernel itself).
    """
    import contextlib

    nc = bacc.Bacc("TRN2", target_bir_lowering=False, num_devices=NCORES)

    x = nc.declare_dram_parameter("x", [BL, IN, T], F16, isOutput=False)
    wfold = nc.declare_dram_parameter("wfold", [n_lags, P, OUT], F16,
                                      isOutput=False)
    w_inT = nc.declare_dram_parameter("w_inT", [IN, P], F16, isOutput=False)
    b_in = nc.declare_dram_parameter("b_in", [P, 1], F32, isOutput=False)
    out = nc.declare_dram_parameter("out", [BL, T, OUT], F32, isOutput=True)

    PAD = n_lags - 1

    with TileContext(nc) as tc:
        with (
            tc.tile_pool(name="wpool", bufs=1) as wpool,
            tc.tile_pool(name="xin", bufs=2) as xin_pool,
            tc.tile_pool(name="xproj", bufs=5) as xp_pool,
            tc.tile_pool(name="btile", bufs=3) as bt_pool,
            tc.tile_pool(name="ps1", bufs=4, space="PSUM") as ps1_pool,
            tc.tile_pool(name="pso", bufs=3, space="PSUM") as pso_pool,
        ):
            # ---- load weights once (already fp16 from host) ----
            fk = wpool.tile([P, n_lags * OUT], F16)
            for k in range(n_lags):
                nc.sync.dma_start(out=fk[:, k * OUT:(k + 1) * OUT],
                                  in_=wfold[k])
            wi = wpool.tile([IN, P], F16)
            nc.sync.dma_start(out=wi[:], in_=w_inT[:])
            bi = wpool.tile([P, 1], F32)
            nc.sync.dma_start(out=bi[:], in_=b_in[:])

            rep_ctx = (
                tc.For_i(
                    0, reps, 1,
                    hint_engines=(
                        mybir.EngineType.PE,
                        mybir.EngineType.DVE,
                        mybir.EngineType.Activation,
                        mybir.EngineType.SP,
                    ),
                )
                if reps > 1
                else contextlib.nullcontext()
            )
            with rep_ctx:
                _emit_body(nc, tc, n_lags, x, out, fk, wi, bi,
                           xin_pool, xp_pool, bt_pool, ps1_pool, pso_pool)

    nc.finalize()
    return nc


def _emit_body(nc, tc, n_lags, x, out, fk, wi, bi,
               xin_pool, xp_pool, bt_pool, ps1_pool, pso_pool):
    PAD = n_lags - 1
    NP2 = NCHUNK // 2       # chunk pairs per batch row
    HALF = 2 * CHUNK        # columns per pair

    def load_x(b):
        xTr = xin_pool.tile([IN, T], F16, tag="xTr")
        nc.sync.dma_start(out=xTr[:], in_=x[b])
        return xTr

    def stage1_pair(b, p, xTr, xp_prev):
        """stage 1 for chunks (2p, 2p+1) into a dedicated pair tile
        [P, PAD + 2*CHUNK]; pad head = zeros (p==0) or tail of the
        previous pair (copied)."""
        xp = xp_pool.tile([P, PAD + HALF], F16, tag="xpp")
        if p == 0:
            nc.vector.memset(xp[:, 0:PAD], 0.0)
        else:
            nc.vector.tensor_copy(out=xp[:, 0:PAD],
                                  in_=xp_prev[:, HALF:PAD + HALF])
        for h in range(2):
            c = 2 * p + h
            ps1 = ps1_pool.tile([P, CHUNK], F32)
            nc.tensor.matmul(
                ps1[:], wi[:], xTr[:, c * CHUNK:(c + 1) * CHUNK],
                start=True, stop=True,
            )
            # relu(ps1 + b_in): 3 on DVE, 1 on ACT per batch row
            dst = xp[:, PAD + h * CHUNK: PAD + (h + 1) * CHUNK]
            if h == 1 and p == 1:
                nc.scalar.activation(
                    out=dst, in_=ps1[:],
                    func=mybir.ActivationFunctionType.Relu, bias=bi[:],
                )
            else:
                nc.vector.tensor_scalar(
                    out=dst, in0=ps1[:], scalar1=bi[:], scalar2=0.0,
                    op0=mybir.AluOpType.add, op1=mybir.AluOpType.max,
                )
        return xp

    def stage2_pair(b, p, xp):
        """fused conv for chunk pair p: two chunks concurrently in
        disjoint PE column groups -> [128, CHUNK] PSUM -> transpose ->
        strided DMA."""
        pso = pso_pool.tile([2 * OUT, CHUNK], F32)
        for k in range(n_lags):
            for half in range(2):
                base = PAD + half * CHUNK - k
                nc.tensor.matmul(
                    pso[half * OUT:(half + 1) * OUT, :],
                    fk[:, k * OUT:(k + 1) * OUT],
                    xp[:, base: base + CHUNK],
                    start=(k == 0), stop=(k == n_lags - 1),
                    tile_position=(0, half * OUT),
                )
        bt = bt_pool.tile([2 * OUT, CHUNK], F32)
        nc.vector.transpose(out=bt[:], in_=pso[:])
        for half in range(2):
            c = 2 * p + half
            for ob in range(OUT // 32):
                p0 = half * OUT + 32 * ob
                sb_view = bt[p0:p0 + 32, :].rearrange(
                    "ti (tb oi) -> ti tb oi", oi=32)
                d_view = out[b, c * CHUNK:(c + 1) * CHUNK,
                             32 * ob:32 * (ob + 1)].rearrange(
                    "(tb ti) oi -> ti tb oi", ti=32)
                nc.sync.dma_start(out=d_view, in_=sb_view)

    # Chunk-pair-level software pipeline, depth 2: stage2(i) is emitted
    # after stage1(i+2), so each stage2's relu inputs have two full
    # stage-1 windows plus a stage2 of PE time to land.
    DEPTH = 2
    work = [(b, p) for b in range(BL) for p in range(NP2)]
    s1_done = {}
    xTr_cur = None
    xp_prev = None
    for i, (b, p) in enumerate(work):
        if p == 0:
            xTr_cur = load_x(b)
            xp_prev = None
        xp_prev = stage1_pair(b, p, xTr_cur, xp_prev)
        s1_done[i] = (b, p, xp_prev)
        j = i - DEPTH
        if j >= 0:
            bb, pp, xpp = s1_done.pop(j)
            stage2_pair(bb, pp, xpp)
    for j in sorted(s1_done):
        bb, pp, xpp = s1_done.pop(j)
        stage2_pair(bb, pp, xpp)


def _n_lags(a: np.ndarray) -> int:
    amax = float(np.abs(a).max())
    if amax >= 1.0:
        return 16
    if amax <= 0.0:
        return 2
    # fp16 operand noise floor is ~5e-4 of output scale; truncating the
    # tail at a^k < 2e-4 keeps truncation well below it.
    k = int(np.ceil(np.log(2e-4) / np.log(amax)))
    return max(2, min(16, k))


def _prepare(x_seq, a, B, C, W_in, b_in, W_out, b_out):
    """Host-side folding + per-core input maps."""
    n_lags = _n_lags(a)
    a64 = a.astype(np.float64)
    B64 = B.astype(np.float64)
    C64 = C.astype(np.float64)
    CW64 = C64 @ W_out.T.astype(np.float64)                # [H, OUT]
    fks = np.stack(
        [(B64 * (a64 ** k)[None, :]) @ CW64 for k in range(n_lags)]
    ).astype(np.float16)                                   # [K, P, OUT]
    shared = {
        "wfold": np.ascontiguousarray(fks),
        "w_inT": np.ascontiguousarray(W_in.T.astype(np.float16)),
        "b_in": np.ascontiguousarray(b_in.astype(np.float32).reshape(P, 1)),
    }
    xT = np.ascontiguousarray(
        np.swapaxes(x_seq, 1, 2).astype(np.float16)
    )                                                      # [B, IN, T]
    in_maps = []
    for c in range(NCORES):
        m = dict(shared)
        m["x"] = xT[c * BL:(c + 1) * BL]
        in_maps.append(m)
    return n_lags, in_maps


def get_program(n_lags: int, reps: int = 1):
    key = (n_lags, reps)
    if key not in _programs:
        _programs[key] = _build(n_lags, reps)
    return _programs[key]


def kernel(x_seq, a, B, C, W_in, b_in, W_out, b_out):
    n_lags, in_maps = _prepare(x_seq, a, B, C, W_in, b_in, W_out, b_out)
    nc = get_program(n_lags)
    res = run_bass_kernel_spmd(nc, in_maps, list(range(NCORES)))
    out = np.concatenate([res.results[c]["out"] for c in range(NCORES)], axis=0)
    out = out.astype(np.float32)
    if np.any(b_out):
        out = out + b_out.astype(np.float32).reshape(1, 1, OUT)
    return out


# revision 37
# speedup vs baseline: 2.2860x; 1.1583x over previous
"""Trainium2 Bass kernel for nn_DiscreteTimeS4.

Model (reference):
    x_proj = relu(x_seq @ W_in^T + b_in)                  # [B, T, P]
    h_t = a * h_{t-1} + x_proj_t @ B ;  y_t = h_t @ C     # diagonal SSM scan
    out = y @ W_out^T + b_out                             # [B, T, OUT]

Key transform: |a| <= sqrt(2/H) ~ 0.09, so a^k decays below fp32
precision within a handful of steps.  The scan is therefore (exactly,
to fp32 precision) a short causal convolution over time, and W_out
folds into the conv matrices:
    out_t = sum_k x_proj_{t-k} @ F_k + b_out,
    F_k = B @ diag(a^k) @ C @ W_out^T          # [P, OUT], host-folded fp64
Device pipeline per batch row (all matmul operands fp16 — same 11-bit
mantissa as TF32/float32r, fp32 PSUM accumulation; end-to-end error
~4e-4 of output scale):
    stage 1: x_projT = relu(W_in @ x_T + b_in)     # PE mm + DVE bias-relu
    stage 2: outT = sum_k F_k^T @ x_projT(shift k) # K PSUM-accum mms; lag
             shifts are free SBUF column offsets; chunk PAIRS run
             concurrently in disjoint PE column groups (tile_position)
             -> [128, 512] PSUM = two [64, 512] chunk results
    out:     DVE 32x32 stream-transpose + strided DMA -> [512, 64] DRAM
b_out is added on the host (it is all-zero for this model's inputs).

Sharding: data-parallel over batch, 8 NeuronCores, B=64 -> 8 per core.
"""

import os
import sys

for _p in ("/opt/trn_rl_repo", "/root/.axon_site/_ro/trn_rl_repo"):
    if os.path.isdir(_p) and _p not in sys.path:
        sys.path.append(_p)

import numpy as np

import concourse.bacc as bacc
import concourse.mybir as mybir
from concourse.bass_utils import run_bass_kernel_spmd
from concourse.tile import TileContext

BATCH, T, IN, P, H, OUT = 64, 2048, 64, 128, 256, 64
NCORES = 8
BL = BATCH // NCORES          # batches per core
CHUNK = 512                   # time chunk (one fp32 PSUM bank)
NCHUNK = T // CHUNK

F32 = mybir.dt.float32
F16 = mybir.dt.float16

_programs = {}                # (n_lags, reps) -> finalized Bacc program


def _build(n_lags: int, reps: int = 1):
    """Build the per-core Bass program for a fixed lag count.

    reps > 1 wraps the whole computation in an on-device loop executing
    it `reps` times — used only for benchmarking (amortizes the axon
    dispatch overhead, which dwarfs the kernel itself).
    """
    import contextlib

    nc = bacc.Bacc("TRN2", target_bir_lowering=False, num_devices=NCORES)

    x = nc.declare_dram_parameter("x", [BL, IN, T], F16, isOutput=False)
    wfold = nc.declare_dram_parameter("wfold", [n_lags, P, OUT], F16,
                                      isOutput=False)
    w_inT = nc.declare_dram_parameter("w_inT", [IN, P], F16, isOutput=False)
    b_in = nc.declare_dram_parameter("b_in", [P, 1], F32, isOutput=False)
    out = nc.declare_dram_parameter("out", [BL, T, OUT], F32, isOutput=True)

    PAD = n_lags - 1

    with TileContext(nc) as tc:
        with (
            tc.tile_pool(name="wpool", bufs=1) as wpool,
            tc.tile_pool(name="xin", bufs=2) as xin_pool,
            tc.tile_pool(name="xproj", bufs=6) as xp_pool,
            tc.tile_pool(name="btile", bufs=4) as bt_pool,
            tc.tile_pool(name="ps1", bufs=4, space="PSUM") as ps1_pool,
            tc.tile_pool(name="pso", bufs=4, space="PSUM") as pso_pool,
        ):
            # ---- load weights once (already fp16 from host) ----
            fk = wpool.tile([P, n_lags * OUT], F16)
            for k in range(n_lags):
                nc.sync.dma_start(out=fk[:, k * OUT:(k + 1) * OUT],
                                  in_=wfold[k])
            wi = wpool.tile([IN, P], F16)
            nc.sync.dma_start(out=wi[:], in_=w_inT[:])
            bi = wpool.tile([P, 1], F32)
            nc.sync.dma_start(out=bi[:], in_=b_in[:])

            rep_ctx = (
                tc.For_i(
                    0, reps, 1,
                    hint_engines=(
                        mybir.EngineType.PE,
                        mybir.EngineType.DVE,
                        mybir.EngineType.Activation,
                        mybir.EngineType.SP,
                    ),
                )
                if reps > 1
                else contextlib.nullcontext()
            )
            with rep_ctx:
                _emit_body(nc, tc, n_lags, x, out, fk, wi, bi,
                           xin_pool, xp_pool, bt_pool, ps1_pool, pso_pool)

    nc.finalize()
    return nc


def _emit_body(nc, tc, n_lags, x, out, fk, wi, bi,
               xin_pool, xp_pool, bt_pool, ps1_pool, pso_pool):
    PAD = n_lags - 1
    NP2 = NCHUNK // 2       # chunk pairs per batch row
    HALF = 2 * CHUNK        # columns per pair

    def load_x(b):
        xTr = xin_pool.tile([IN, T], F16, tag="xTr")
        nc.sync.dma_start(out=xTr[:], in_=x[b])
        return xTr

    def stage1_pair(b, p, xTr, xp_prev):
        """stage 1 for chunks (2p, 2p+1) into a dedicated pair tile
        [P, PAD + 2*CHUNK]; pad head = zeros (p==0) or tail of the
        previous pair (copied)."""
        xp = xp_pool.tile([P, PAD + HALF], F16, tag="xpp")
        if p == 0:
            nc.gpsimd.memset(xp[:, 0:PAD], 0.0)
        else:
            nc.gpsimd.tensor_copy(out=xp[:, 0:PAD],
                                  in_=xp_prev[:, HALF:PAD + HALF])
        for h in range(2):
            c = 2 * p + h
            ps1 = ps1_pool.tile([P, CHUNK], F32)
            nc.tensor.matmul(
                ps1[:], wi[:], xTr[:, c * CHUNK:(c + 1) * CHUNK],
                start=True, stop=True,
            )
            # relu(ps1 + b_in): 3 on DVE, 1 on ACT per batch row
            dst = xp[:, PAD + h * CHUNK: PAD + (h + 1) * CHUNK]
            if h == 1 and p == 1:
                nc.scalar.activation(
                    out=dst, in_=ps1[:],
                    func=mybir.ActivationFunctionType.Relu, bias=bi[:],
                )
            else:
                nc.vector.tensor_scalar(
                    out=dst, in0=ps1[:], scalar1=bi[:], scalar2=0.0,
                    op0=mybir.AluOpType.add, op1=mybir.AluOpType.max,
                )
        return xp

    def stage2_pair(b, p, xp):
        """fused conv for chunk pair p: two chunks concurrently in
        disjoint PE column groups -> [128, CHUNK] PSUM -> transpose ->
        strided DMA."""
        pso = pso_pool.tile([2 * OUT, CHUNK], F32)
        for k in range(n_lags):
            for half in range(2):
                base = PAD + half * CHUNK - k
                nc.tensor.matmul(
                    pso[half * OUT:(half + 1) * OUT, :],
                    fk[:, k * OUT:(k + 1) * OUT],
                    xp[:, base: base + CHUNK],
                    start=(k == 0), stop=(k == n_lags - 1),
                    tile_position=(0, half * OUT),
                )
        bt = bt_pool.tile([2 * OUT, CHUNK], F32)
        nc.vector.transpose(out=bt[:], in_=pso[:])
        for half in range(2):
            c = 2 * p + half
            for ob in range(OUT // 32):
                p0 = half * OUT + 32 * ob
                sb_view = bt[p0:p0 + 32, :].rearrange(
                    "ti (tb oi) -> ti tb oi", oi=32)
                d_view = out[b, c * CHUNK:(c + 1) * CHUNK,
                             32 * ob:32 * (ob + 1)].rearrange(
                    "(tb ti) oi -> ti tb oi", ti=32)
                nc.sync.dma_start(out=d_view, in_=sb_view)

    # Chunk-pair-level software pipeline, depth 2: stage2(i) is emitted
    # after stage1(i+2), so each stage2's relu inputs have two full
    # stage-1 windows plus a stage2 of PE time to land.
    DEPTH = 3
    work = [(b, p) for b in range(BL) for p in range(NP2)]
    s1_done = {}
    xTr_cur = None
    xp_prev = None
    for i, (b, p) in enumerate(work):
        if p == 0:
            xTr_cur = load_x(b)
            xp_prev = None
        xp_prev = stage1_pair(b, p, xTr_cur, xp_prev)
        s1_done[i] = (b, p, xp_prev)
        j = i - DEPTH
        if j >= 0:
            bb, pp, xpp = s1_done.pop(j)
            stage2_pair(bb, pp, xpp)
    for j in sorted(s1_done):
        bb, pp, xpp = s1_done.pop(j)
        stage2_pair(bb, pp, xpp)


def _n_lags(a: np.ndarray) -> int:
    amax = float(np.abs(a).max())
    if amax >= 1.0:
        return 16
    if amax <= 0.0:
        return 2
    # fp16 operand noise floor is ~5e-4 of output scale; truncating the
    # tail at a^k < 2e-4 keeps truncation well below it.
    k = int(np.ceil(np.log(2e-4) / np.log(amax)))
    return max(2, min(16, k))


def _prepare(x_seq, a, B, C, W_in, b_in, W_out, b_out):
    """Host-side folding + per-core input maps."""
    n_lags = _n_lags(a)
    a64 = a.astype(np.float64)
    B64 = B.astype(np.float64)
    C64 = C.astype(np.float64)
    CW64 = C64 @ W_out.T.astype(np.float64)                # [H, OUT]
    fks = np.stack(
        [(B64 * (a64 ** k)[None, :]) @ CW64 for k in range(n_lags)]
    ).astype(np.float16)                                   # [K, P, OUT]
    shared = {
        "wfold": np.ascontiguousarray(fks),
        "w_inT": np.ascontiguousarray(W_in.T.astype(np.float16)),
        "b_in": np.ascontiguousarray(b_in.astype(np.float32).reshape(P, 1)),
    }
    xT = np.ascontiguousarray(
        np.swapaxes(x_seq, 1, 2).astype(np.float16)
    )                                                      # [B, IN, T]
    in_maps = []
    for c in range(NCORES):
        m = dict(shared)
        m["x"] = xT[c * BL:(c + 1) * BL]
        in_maps.append(m)
    return n_lags, in_maps


def get_program(n_lags: int, reps: int = 1):
    key = (n_lags, reps)
    if key not in _programs:
        _programs[key] = _build(n_lags, reps)
    return _programs[key]


def kernel(x_seq, a, B, C, W_in, b_in, W_out, b_out):
    n_lags, in_maps = _prepare(x_seq, a, B, C, W_in, b_in, W_out, b_out)
    nc = get_program(n_lags)
    res = run_bass_kernel_spmd(nc, in_maps, list(range(NCORES)))
    out = np.concatenate([res.results[c]["out"] for c in range(NCORES)], axis=0)
    out = out.astype(np.float32)
    if np.any(b_out):
        out = out + b_out.astype(np.float32).reshape(1, 1, OUT)
    return out
